# revision 22
# baseline (speedup 1.0000x reference)
"""BaiChuan attention layer on 8 Trainium2 NeuronCores.

Sharding: tensor-parallel over heads within groups of 4 cores (W_pack
column-parallel, o_proj column-parallel after per-head-pair AllGathers
of attention outputs), data-parallel over the batch across the groups.

v3: q/k projection in fp8-e4m3 with DoubleRow matmuls (K=256 per MM,
inputs pre-scaled x64 on host, rescaled 2^-12 on the PSUM-evacuate
copy); v projected in bf16 directly into natural [token, hd] layout
(SBUF-resident, no PE transposes); causal mask applied as a -1e4
PSUM-init via identity matmul (off the DVE); softmax denominator as
accumulated [1,512] ones-matmuls on the PE + reciprocal_approx_fast;
AllGathers batched per head-pair (4 ops) in bf16; o_proj split as
heads 0-5 main + heads 6-7 increment.

Per-core dataflow (core c: batch b=c//4, rank r=c%4, heads 8r..8r+8):
  stage A-qk: for th token-block: qkT[jt] = Wqk_jt @ hs_th   (fp8 DR)
  stage A-v:  for tb 128-token block: v[tb] = hs_tb.T @ Wv   (bf16)
  stage B: per head: neox RoPE on qT,kT (DVE), causal attention with
           sT = kT-blocks.T @ qT, batched exp on ACT, PV from resident
           v, pair AllGathers overlapping later heads' compute.
  stage C: o_proj column-parallel over the gathered head dim.
"""
import sys
sys.path.insert(0, '/opt/trn_rl_repo')
import numpy as np
import ml_dtypes

import concourse.bass as bass
from concourse import bacc
import concourse.mybir as mybir
from concourse.tile import TileContext
from concourse.bass_utils import run_bass_kernel_spmd
from concourse.masks import make_identity

f32 = mybir.dt.float32
bf16 = mybir.dt.bfloat16
fp8 = mybir.dt.float8e4
AF = mybir.ActivationFunctionType
DR = mybir.MatmulPerfMode.DoubleRow

B, S, H, NH = 2, 2048, 4096, 32
HD = H // NH                    # 128
THETA = 10000.0
NCORES, TPN = 8, 4              # 2 groups of 4 (DP over batch x TP over heads)
HPC = NH // TPN                 # 8 heads per core
JC = HPC * HD                   # 1024 per-core q (=k=v) width
SCALE = HD ** -0.5
FP8_S = 64.0                    # host pre-scale on hs8/wqk8
UNSCALE = 1.0 / (FP8_S * FP8_S)  # 2^-12 rescale on the qk PSUM evacuate
GROUPS = [[0, 1, 2, 3], [4, 5, 6, 7]]
NIB = H // 128                  # 32 contraction blocks
NJT = 2 * HPC                   # 16 q/k row-tiles in stage A (k0,q0,k1,q1..)
NG = S // 512                   # 4 query blocks per head
NKB = S // 128                  # 16 key blocks per head
NTB = NKB                       # 16 token 128-blocks
AG_GROUPS = [[0, 1], [2, 3], [4, 5, 6], [7]]   # AllGather head batches
NJB_MAIN = 7 * TPN              # heads 0-6 -> 28 o_proj jb blocks
NJB_TAIL = TPN                  # head 7 -> 4 jb blocks


def build_nc():
    nc = bacc.Bacc(None)
    hsT = nc.declare_dram_parameter("hsT", [H, S], bf16, isOutput=False)
    hs8 = nc.declare_dram_parameter("hs8", [H, S], fp8, isOutput=False)
    wqk8 = nc.declare_dram_parameter("wqk8", [H, NJT * 128], fp8,
                                     isOutput=False)
    wvT = nc.declare_dram_parameter("wvT", [H, JC], bf16, isOutput=False)
    woT = nc.declare_dram_parameter("woT", [H, JC], bf16, isOutput=False)
    cosf = nc.declare_dram_parameter("cosf", [HD, S], bf16, isOutput=False)
    sinm = nc.declare_dram_parameter("sinm", [HD, S], bf16, isOutput=False)
    lmask = nc.declare_dram_parameter("lmask", [4, 128, 512], bf16,
                                      isOutput=False)
    out = nc.declare_dram_parameter("out", [S, JC], f32, isOutput=True)

    qk_d = [nc.dram_tensor(f"qk_d{j}", [128, S], bf16) for j in range(NJT)]
    attn_d = nc.dram_tensor("attn_d", [HPC, HD, S], bf16)
    attn_ag = [nc.dram_tensor(f"attn_ag{i}", [TPN * len(g) * HD, S], bf16)
               for i, g in enumerate(AG_GROUPS)]

    hsT_v = hsT[:].rearrange("(n p) t -> p n t", p=128)      # [128, 32, S]
    hs8_v = hs8[:].rearrange("(n p) t -> p n t", p=128)      # [128, 32, S]
    wqk8_v = wqk8[:].rearrange("(n p) j -> p n j", p=128)    # [128, 32, 2048]
    wvT_v = wvT[:].rearrange("(n p) j -> p n j", p=128)      # [128, 32, JC]
    woT_v = woT[:].rearrange("(n p) m -> p n m", p=128)      # [128, 32, JC]
    ag_views = [attn_ag[i][:].rearrange("(x q) t -> q x t", q=128)
                for i in range(4)]                       # [128, 4*len(g), S]

    with TileContext(nc) as tc:
        # persistent pool: resident v (natural layout) + stage-B prep tiles
        with tc.tile_pool(name="P0", bufs=1) as p0:
            v_sb = p0.tile([128, NTB, JC], bf16, tag="v_sb", bufs=1)

            # wv weight pool spans both stage-A passes only
            with tc.tile_pool(name="PAW", bufs=1) as paw:
                wv_sb = paw.tile([128, NIB, JC], bf16, tag="wv", bufs=1)

                # ------------- stage A-qk: q/k projection (fp8 DR) ----------
                with nc.named_scope("stageAqk"), \
                     tc.tile_pool(name="stA", bufs=1) as pa, \
                     tc.tile_pool(name="psA", bufs=6, space="PSUM") as psA:
                    hs_tiles = {}

                    def alloc_hs8(th):
                        hs_tiles[th] = pa.tile([128, NIB, 512], fp8,
                                               tag="hs", bufs=2,
                                               name=f"hs_{th}")

                    def load_hs8_chunk(th, d):
                        nc.sync.dma_start(
                            out=hs_tiles[th][:, 8 * d:8 * (d + 1), :],
                            in_=hs8_v[:, 8 * d:8 * (d + 1),
                                      th * 512:(th + 1) * 512])

                    alloc_hs8(0)
                    for d in range(4):
                        load_hs8_chunk(0, d)
                    for th in range(S // 512):
                        hs_th = hs_tiles.pop(th)
                        for jt in range(NJT):
                            w_sb = pa.tile([128, NIB, 128], fp8, tag="w",
                                           bufs=3, name=f"w_{th}_{jt}")
                            nc.sync.dma_start(
                                out=w_sb[:],
                                in_=wqk8_v[:, :, jt * 128:(jt + 1) * 128])
                            # prefetches trickle in between the w loads
                            if jt in (4, 6, 8, 10) and th + 1 < S // 512:
                                if jt == 4:
                                    alloc_hs8(th + 1)
                                load_hs8_chunk(th + 1, jt // 2 - 2)
                            if jt in (12, 14):
                                d = 2 * th + (jt - 12) // 2
                                nc.sync.dma_start(
                                    out=wv_sb[:, 4 * d:4 * (d + 1), :],
                                    in_=wvT_v[:, 4 * d:4 * (d + 1), :])
                            ps = psA.tile([128, 512], f32, tag="psA",
                                          name=f"psA_{th}_{jt}")
                            for i2 in range(NIB // 2):
                                nc.tensor.matmul(
                                    ps[:], w_sb[:, 2 * i2:2 * i2 + 2, :],
                                    hs_th[:, 2 * i2:2 * i2 + 2, :],
                                    start=(i2 == 0),
                                    stop=(i2 == NIB // 2 - 1),
                                    perf_mode=DR)
                            st = pa.tile([128, 512], bf16, tag="oA", bufs=4,
                                         name=f"stA_{th}_{jt}")
                            nc.scalar.mul(st[:], ps[:], UNSCALE)
                            nc.sync.dma_start(
                                out=qk_d[jt][:][:, th * 512:(th + 1) * 512],
                                in_=st[:])

                # stage-B prep at P0 level: loads overlap stage A-v
                cos_sb = p0.tile([128, S], bf16, tag="cos", bufs=1)
                sin_sb = p0.tile([128, S], bf16, tag="sin", bufs=1)
                lm_sb = p0.tile([128, 4, 512], bf16, tag="lmask", bufs=1)
                ident = p0.tile([128, 128], bf16, tag="ident", bufs=1)
                ones_b = p0.tile([128, 1], bf16, tag="ones", bufs=1)

                def load_rope(jt, tag, h):
                    """load qk_d row-block jt, apply neox rope (bf16)"""
                    raw = p0.tile([128, S], bf16, tag="raw", bufs=6,
                                  name=f"{tag}_raw_{h}")
                    nc.sync.dma_start(out=raw[:], in_=qk_d[jt][:])
                    sw = p0.tile([128, S], bf16, tag="raw", bufs=6,
                                 name=f"{tag}_sw_{h}")
                    nc.sync.dma_start(out=sw[0:64, :],
                                      in_=qk_d[jt][:][64:128, :])
                    nc.sync.dma_start(out=sw[64:128, :],
                                      in_=qk_d[jt][:][0:64, :])
                    t1 = p0.tile([128, S], bf16, tag="ropetmp", bufs=2,
                                 name=f"{tag}_t1_{h}")
                    t2 = p0.tile([128, S], bf16, tag="ropetmp", bufs=2,
                                 name=f"{tag}_t2_{h}")
                    with tc.high_priority():
                        nc.vector.tensor_mul(t1[:], raw[:], cos_sb[:])
                        nc.vector.tensor_mul(t2[:], sw[:], sin_sb[:])
                        rt = p0.tile([128, S], bf16, tag=f"{tag}_r",
                                     bufs=2, name=f"{tag}_roped_{h}")
                        nc.vector.tensor_add(rt[:], t1[:], t2[:])
                    return rt

                kts, qts = {}, {}

                def prep(h):
                    kts[h] = load_rope(2 * h, "kr", h)
                    qts[h] = load_rope(2 * h + 1, "qr", h)

                # ------------- stage A-v: v in natural layout (bf16) --------
                with nc.named_scope("stageAv"), \
                     tc.tile_pool(name="stV", bufs=1) as pv, \
                     tc.tile_pool(name="psV", bufs=6, space="PSUM") as psV:
                    hsv_tiles = {}

                    def load_hsv(tb):
                        t = pv.tile([128, NIB, 128], bf16, tag="hsv", bufs=3,
                                    name=f"hsv_{tb}")
                        for d in range(2):
                            nc.sync.dma_start(
                                out=t[:, 16 * d:16 * (d + 1), :],
                                in_=hsT_v[:, 16 * d:16 * (d + 1),
                                          tb * 128:(tb + 1) * 128])
                        hsv_tiles[tb] = t

                    load_hsv(0)
                    load_hsv(1)
                    # prep-tile fills + head-0/1 rope run during stage A-v
                    nc.sync.dma_start(out=cos_sb[:], in_=cosf[:])
                    nc.sync.dma_start(out=sin_sb[:], in_=sinm[:])
                    nc.sync.dma_start(out=lm_sb[:],
                                      in_=lmask[:].rearrange("v p x -> p v x"))
                    make_identity(nc, ident[:])
                    nc.vector.memset(ones_b[:], 1.0)
                    prep(0)
                    prep(1)
                    for tb in range(NTB):
                        if tb + 2 < NTB:
                            load_hsv(tb + 2)
                        hs_tb = hsv_tiles.pop(tb)
                        for mc in range(JC // 512):
                            ps = psV.tile([128, 512], f32, tag="psV",
                                          name=f"psV_{tb}_{mc}")
                            for ib in range(NIB):
                                nc.tensor.matmul(
                                    ps[:], hs_tb[:, ib, :],
                                    wv_sb[:, ib, mc * 512:(mc + 1) * 512],
                                    start=(ib == 0), stop=(ib == NIB - 1))
                            nc.vector.tensor_copy(
                                v_sb[:, tb, mc * 512:(mc + 1) * 512], ps[:])

            # ------------- stages B+C share the o_proj weight pool ----------
            with tc.tile_pool(name="stWo", bufs=1, side="right") as pwo:
                wo_h0 = pwo.tile([128, NIB, JC // 2], bf16, tag="wo0", bufs=1)

                # ------------- stage B: rope + causal attention -------------
                wo_h1 = pwo.tile([128, NIB, JC // 2], bf16, tag="wo1",
                                 bufs=1)
                with nc.named_scope("stageB"), \
                     tc.tile_pool(name="stB", bufs=1) as pb, \
                     tc.tile_pool(name="psB", bufs=1, space="PSUM") as psB:
                    for d in range(8):
                        nc.sync.dma_start(
                            out=wo_h0[:, 4 * d:4 * (d + 1), :],
                            in_=woT_v[:, 4 * d:4 * (d + 1), :JC // 2])

                    for h in range(HPC):
                        with nc.named_scope(f"head{h}"):
                            if h + 2 < HPC:
                                prep(h + 2)
                            # wo second half streams in behind the rope loads
                            nc.sync.dma_start(
                                out=wo_h1[:, 4 * h:4 * (h + 1), :],
                                in_=woT_v[:, 4 * h:4 * (h + 1), JC // 2:])
                            kT, qT = kts.pop(h), qts.pop(h)
                            hc0 = h * 128
                            attn = pb.tile([128, S], bf16, tag="attn", bufs=2,
                                           name=f"attn_{h}")
                            for g in range(NG):
                                nu = 2 * g + 2      # 1024-wide units
                                nkb = 2 * nu
                                po = psB.tile([128, 512], f32, tag="po",
                                              bufs=2, name=f"po_{h}_{g}")
                                pden = psB.tile([1, 512], f32, tag="pden",
                                                bufs=2, name=f"pden_{h}_{g}")
                                for u in range(nu):
                                    ps2 = psB.tile([128, 1024], f32,
                                                   tag="pss", bufs=2,
                                                   name=f"pss_{h}_{g}_{u}")
                                    diag = u >= 2 * g
                                    for half in range(2):
                                        kb = 2 * u + half
                                        dst = ps2[:, half * 512:
                                                  (half + 1) * 512]
                                        if diag:
                                            mi = 2 * (u - 2 * g) + half
                                            nc.tensor.matmul(
                                                dst, ident[:],
                                                lm_sb[:, mi, :],
                                                start=True, stop=False)
                                        nc.tensor.matmul(
                                            dst,
                                            kT[:, kb * 128:(kb + 1) * 128],
                                            qT[:, g * 512:(g + 1) * 512],
                                            start=not diag, stop=True)
                                    pt = pb.tile([128, 1024], bf16, tag="pt",
                                                 bufs=3,
                                                 name=f"pt_{h}_{g}_{u}")
                                    nc.scalar.activation(pt[:], ps2[:],
                                                         AF.Exp, scale=SCALE)
                                    for half in range(2):
                                        kb = 2 * u + half
                                        ph = pt[:, half * 512:
                                                (half + 1) * 512]
                                        nc.tensor.matmul(
                                            pden[:], ones_b[:], ph,
                                            start=(kb == 0),
                                            stop=(kb == nkb - 1))
                                        nc.tensor.matmul(
                                            po[:],
                                            v_sb[:, kb, hc0:hc0 + 128], ph,
                                            start=(kb == 0),
                                            stop=(kb == nkb - 1))
                                den1 = pb.tile([1, 512], f32, tag="den1",
                                               bufs=2, name=f"den1_{h}_{g}")
                                nc.scalar.copy(den1[:], pden[:])
                                rd1 = pb.tile([1, 512], f32, tag="rd1",
                                              bufs=2, name=f"rd1_{h}_{g}")
                                nc.vector.reciprocal_approx_fast(
                                    out=rd1[:], in_=den1[:])
                                rden = pb.tile([128, 512], f32, tag="rden",
                                               bufs=2, name=f"rden_{h}_{g}")
                                nc.gpsimd.partition_broadcast(rden[:], rd1[:])
                                nc.vector.tensor_mul(
                                    attn[:, g * 512:(g + 1) * 512], po[:],
                                    rden[:])
                            nc.sync.dma_start(out=attn_d[:][h], in_=attn[:])
                            for i, grp in enumerate(AG_GROUPS[:-1]):
                                if h == grp[-1]:
                                    nc.gpsimd.collective_compute(
                                        "AllGather", mybir.AluOpType.bypass,
                                        replica_groups=GROUPS,
                                        ins=[attn_d[:][grp[0]:grp[-1] + 1]],
                                        outs=[attn_ag[i][:]])

                # head-7 AllGather outside the stage-B pool scope
                nc.gpsimd.collective_compute(
                    "AllGather", mybir.AluOpType.bypass, replica_groups=GROUPS,
                    ins=[attn_d[:][HPC - 1:HPC]], outs=[attn_ag[3][:]])

                # ------ stage C: o_proj, pairs 0-2 main + pair-3 increment ---
                with nc.named_scope("stageC"), \
                     tc.tile_pool(name="stC", bufs=1) as pc, \
                     tc.tile_pool(name="psC", bufs=4, space="PSUM") as psC:
                    for tb in range(NKB):
                        at_sb = pc.tile([128, NJB_MAIN, 128], bf16, tag="atC",
                                        bufs=3, name=f"atC_{tb}")
                        ncol = 0
                        for i in range(3):
                            w = 4 * len(AG_GROUPS[i])
                            nc.sync.dma_start(
                                out=at_sb[:, ncol:ncol + w, :],
                                in_=ag_views[i][:, :,
                                                tb * 128:(tb + 1) * 128])
                            ncol += w
                        a7_sb = pc.tile([128, NJB_TAIL, 128], bf16, tag="a7C",
                                        bufs=3, name=f"a7C_{tb}")
                        nc.sync.dma_start(
                            out=a7_sb[:],
                            in_=ag_views[3][:, :, tb * 128:(tb + 1) * 128])
                        for mc in range(JC // 512):
                            wsrc = wo_h0 if mc == 0 else wo_h1
                            psc = psC.tile([128, 512], f32, tag="psC",
                                           name=f"psC_{tb}_{mc}")
                            for jb in range(NJB_MAIN):
                                nc.tensor.matmul(
                                    psc[:], at_sb[:, jb, :], wsrc[:, jb, :],
                                    start=(jb == 0), stop=(jb == NJB_MAIN - 1))
                            main_sb = pc.tile([128, 512], f32, tag="mainC",
                                              bufs=4, name=f"mainC_{tb}_{mc}")
                            nc.scalar.copy(main_sb[:], psc[:])
                            ps7 = psC.tile([128, 512], f32, tag="ps7", bufs=2,
                                           name=f"ps7_{tb}_{mc}")
                            for i in range(NJB_TAIL):
                                nc.tensor.matmul(
                                    ps7[:], a7_sb[:, i, :],
                                    wsrc[:, NJB_MAIN + i, :],
                                    start=(i == 0), stop=(i == NJB_TAIL - 1))
                            oc = pc.tile([128, 512], f32, tag="oC", bufs=4,
                                         name=f"oC_{tb}_{mc}")
                            nc.vector.tensor_add(oc[:], ps7[:], main_sb[:])
                            nc.sync.dma_start(
                                out=out[:][tb * 128:(tb + 1) * 128,
                                           mc * 512:(mc + 1) * 512],
                                in_=oc[:])

    nc.finalize()
    return nc


_NC_CACHE = None


def _get_nc():
    global _NC_CACHE
    if _NC_CACHE is None:
        _NC_CACHE = build_nc()
    return _NC_CACHE


def _host_inputs(hidden_states, positions, w_pack, w_o):
    hidden_states = np.asarray(hidden_states, dtype=np.float32)
    positions = np.asarray(positions)
    w_pack = np.asarray(w_pack, dtype=np.float32)
    w_o = np.asarray(w_o, dtype=np.float32)

    half = HD // 2
    inv_freq = (1.0 / (THETA ** (np.arange(half, dtype=np.float32) / half)))

    # -1e4 log-mask for the 4 diagonal (128x512) tiles of a q-block
    lmask = np.zeros((4, 128, 512), dtype=np.float32)
    xs = np.arange(512)[None, :]
    ps = np.arange(128)[:, None]
    for v in range(4):
        lmask[v] = np.where(xs >= ps + 128 * v, 0.0, -1e4)

    in_maps = []
    for c in range(NCORES):
        b, r = divmod(c, TPN)
        heads = np.arange(HPC * r, HPC * (r + 1))
        rows = (heads[:, None] * HD + np.arange(HD)[None, :]).reshape(-1)
        Wq = w_pack[rows]                                        # [JC, H]
        Wk = w_pack[H + rows]
        Wv = w_pack[2 * H + rows]
        # qk weight columns interleaved per head: k_h then q_h
        wqk = np.empty((NJT * 128, H), dtype=np.float32)
        for h in range(HPC):
            wqk[256 * h:256 * h + 128] = Wk[128 * h:128 * (h + 1)]
            wqk[256 * h + 128:256 * (h + 1)] = Wq[128 * h:128 * (h + 1)]
        wqk8 = np.ascontiguousarray(wqk.T) * FP8_S               # [H, 2048]
        wvT = np.ascontiguousarray(Wv.T)                         # [H, JC]
        # o_proj m-shard rows, j-order permuted to match the batched
        # AllGather layout: group-major, then rank, then head-in-group
        wo_shard = w_o[JC * r:JC * (r + 1), :]                   # [JC, H]
        woT_full = np.ascontiguousarray(wo_shard.T)              # [H=j, JC]
        order = [HPC * rr + h
                 for g in AG_GROUPS for rr in range(TPN) for h in g]
        woT_perm = woT_full.reshape(NH, HD, JC)[order].reshape(H, JC)
        hsT = np.ascontiguousarray(hidden_states[b].T)           # [H, S]
        ang = positions[b].astype(np.float32)[None, :] * inv_freq[:, None]
        cos_t = np.cos(ang).astype(np.float32)                   # [64, S]
        sin_t = np.sin(ang).astype(np.float32)
        cosf = np.concatenate([cos_t, cos_t], axis=0)            # [128, S]
        sinm = np.concatenate([-sin_t, sin_t], axis=0)
        in_maps.append({
            "hsT": hsT.astype(ml_dtypes.bfloat16),
            "hs8": (hsT * FP8_S).astype(ml_dtypes.float8_e4m3fn),
            "wqk8": wqk8.astype(ml_dtypes.float8_e4m3fn),
            "wvT": wvT.astype(ml_dtypes.bfloat16),
            "woT": np.ascontiguousarray(woT_perm).astype(ml_dtypes.bfloat16),
            "cosf": cosf.astype(ml_dtypes.bfloat16),
            "sinm": sinm.astype(ml_dtypes.bfloat16),
            "lmask": lmask.astype(ml_dtypes.bfloat16),
        })
    return in_maps


def kernel(hidden_states, positions, w_pack, w_o):
    import os
    os.environ["BASS_NEVER_TRACE"] = "1"
    nc = _get_nc()
    in_maps = _host_inputs(hidden_states, positions, w_pack, w_o)
    res = run_bass_kernel_spmd(nc, in_maps, list(range(NCORES)))
    out = np.empty((B, S, H), dtype=np.float32)
    for c in range(NCORES):
        b, r = divmod(c, TPN)
        out[b][:, JC * r:JC * (r + 1)] = res.results[c]["out"]
    return out
